# revision 1
# baseline (speedup 1.0000x reference)
"""Trainium2 Bass kernel for AssignClsLabel (clipped-IoU >= 0.7 proposal labeling).

Problem: bboxess [8, 65536, 4] f32, gt_bboxess [8, 64, 4] f32,
gt_counts/counts [8,1] int. Output labels [8, 65536, 1] int (0/1).

Sharding: data-parallel, one batch per NeuronCore (8 cores).

Device math (validated bit-exact vs reference on the fixed dataset):
  per (proposal n, gt a):
    r2(y) = relu(d21 - relu(y - gy1))      [= gy2 - clip(y, gy1, gy2)]
    dy = r2(y1) - r2(y2)  (= clip(y2)-clip(y1));  dx likewise with gx
    inter = dy*dx
    u = (area + ga) - inter
    m = (0.7*u - inter) * u       # sign test: iou >= 0.7  <=>  m <= 0
  label = (min_a m <= 0) & (n < count)
Invalid gts (a >= gt_count) are replaced host-side by a far box (2,2,3,3)
which can never fire.

Engine split: ScalarE (ACT) computes ALL clips in relu form (2 ops per
coordinate pair per gt, FD=1024 over the raw interleaved (y1,y2)/(x1,x2)
pairs); VectorE (DVE) does the whole arithmetic core (dy, dx, inter, u,
dp, m, acc-min). ACT and DVE overlap perfectly on TRN2; GPSIMD is avoided
for bulk elementwise (it serializes with DVE on the SBUF ports).
"""
import sys

import numpy as np

if "/opt/trn_rl_repo" not in sys.path:
    sys.path.insert(0, "/opt/trn_rl_repo")

import concourse.mybir as mybir
import concourse.tile as tile
from concourse import bacc
from concourse.bass_utils import run_bass_kernel_spmd

AOP = mybir.AluOpType
ACT = mybir.ActivationFunctionType
F32 = mybir.dt.float32
I32 = mybir.dt.int32

P = 128          # SBUF partitions; proposals n = p*C + c
A = 64           # gt boxes per batch
G = 4            # gts per inner group
BIG = 3.0e38

# scal column layout (all values broadcast to 128 partitions host-side)
COL_GA = 0       # ga = (gy2-gy1)*(gx2-gx1)
COL_NGY1 = 64    # -gy1  (ACT bias for relu(y - gy1))
COL_D21Y = 128   # gy2 - gy1
COL_NGX1 = 192   # -gx1
COL_E21X = 256   # gx2 - gx1
COL_GY1 = 320    # raw bounds (DVE-direct clip path, group 0 ramp)
COL_GY2 = 384
COL_GX1 = 448
COL_GX2 = 512
COL_CNT = 576
SCAL_W = 580


def build_graph(C: int):
    """One-core graph; SPMD across 8 cores. C = proposals per partition."""
    NG = A // G
    FD = G * C
    # Bacc (not plain Bass): its finalize() runs generate_event_semaphores,
    # which splits multi-proc sync waits into EventSemaphore instructions —
    # walrus accepts at most ONE wait per ordinary instruction.
    nc = bacc.Bacc()

    bbox_d = nc.declare_dram_parameter("bbox", [P, 4 * C], F32, isOutput=False)
    scal_d = nc.declare_dram_parameter("scal", [P, SCAL_W], F32, isOutput=False)
    iota_d = nc.declare_dram_parameter("iota", [P, C], F32, isOutput=False)
    out_d = nc.declare_dram_parameter("out", [P, 2 * C], I32, isOutput=True)

    with tile.TileContext(nc) as tc:
        with (
            tc.tile_pool(name="persist", bufs=1) as pp,
            tc.tile_pool(name="grp", bufs=1) as gp,
        ):
            braw = pp.tile([P, 4 * C], F32, tag="braw")
            scal = pp.tile([P, SCAL_W], F32, tag="scal")
            iot = pp.tile([P, C], F32, tag="iot")
            nc.sync.dma_start(braw[:], bbox_d[:])
            nc.sync.dma_start(scal[:], scal_d[:])
            nc.sync.dma_start(iot[:], iota_d[:])

            # raw free idx = 4c + 2j + i : i=0 -> y coords, i=1 -> x coords;
            # j=0 -> (y1,x1), j=1 -> (y2,x2)
            bv = braw[:].rearrange("p (c j i) -> p c j i", j=2, i=2)
            ypair = bv[:, :, :, 0]          # [P, C, 2] = (y1, y2), steps (4,2)
            xpair = bv[:, :, :, 1]          # [P, C, 2] = (x1, x2)
            y1v, y2v = bv[:, :, 0, 0], bv[:, :, 1, 0]
            x1v, x2v = bv[:, :, 0, 1], bv[:, :, 1, 1]

            area = pp.tile([P, C], F32, tag="area")
            tdy = pp.tile([P, C], F32, tag="tdy")
            nc.vector.tensor_tensor(tdy[:], y2v, y1v, AOP.subtract)
            nc.vector.tensor_tensor(area[:], x2v, x1v, AOP.subtract)
            nc.vector.tensor_tensor(area[:], tdy[:], area[:], AOP.mult)

            acc = [pp.tile([P, FD], F32, tag=f"acc{i}", name=f"acc{i}")
                   for i in range(2)]
            nc.vector.memset(acc[0][:], BIG)

            # ACT's per-instruction sync-wait budget is 1: give it DVE-produced
            # scalars so its data deps collapse onto the DVE proc.
            scal2 = pp.tile([P, SCAL_W], F32, tag="scal2")
            nc.vector.tensor_copy(scal2[:], scal[:])

            def gcol(base, a):
                return scal2[:, base + a : base + a + 1]

            # tail prep, no dependence on the group loop
            vb = pp.tile([P, C], F32, tag="vb")
            nc.vector.tensor_scalar(
                vb[:], iot[:], scal[:, COL_CNT:COL_CNT + 1], None, AOP.is_lt)
            outsb = pp.tile([P, 2 * C], I32, tag="outsb")
            nc.vector.memset(outsb[:], 0)

            for g in range(NG):
                pairs = []
                for j in range(G):
                    a = g * G + j
                    if g == 0 and j < 2:
                        # DVE-direct clips so DVE has work during ACT's ramp;
                        # dedicated tags to keep ACT's slots free.
                        yyc = gp.tile([P, 2 * C], F32, tag="yycd", bufs=1,
                                      name=f"yyc_{j}")
                        xxc = gp.tile([P, 2 * C], F32, tag="xxcd", bufs=1,
                                      name=f"xxc_{j}")
                        yycv = yyc[:].rearrange("p (c j) -> p c j", j=2)
                        xxcv = xxc[:].rearrange("p (c j) -> p c j", j=2)
                        nc.vector.tensor_scalar(
                            yycv, ypair, gcol(COL_GY1, a), gcol(COL_GY2, a),
                            AOP.max, AOP.min)
                        nc.vector.tensor_scalar(
                            xxcv, xpair, gcol(COL_GX1, a), gcol(COL_GX2, a),
                            AOP.max, AOP.min)
                        pairs.append((yyc, xxc, True))
                        continue
                    # ACT: paired relu clips, FD=1024 per op
                    r1p = gp.tile([P, 2 * C], F32, tag="r1p", bufs=2)
                    r2p = gp.tile([P, 2 * C], F32, tag="r2p", bufs=2,
                                  name=f"r2p_{g}_{j}")
                    s1p = gp.tile([P, 2 * C], F32, tag="s1p", bufs=2)
                    s2p = gp.tile([P, 2 * C], F32, tag="s2p", bufs=2,
                                  name=f"s2p_{g}_{j}")
                    r1pv = r1p[:].rearrange("p (c j) -> p c j", j=2)
                    s1pv = s1p[:].rearrange("p (c j) -> p c j", j=2)
                    nc.scalar.activation(
                        r1pv, ypair, ACT.Relu, bias=gcol(COL_NGY1, a))
                    nc.scalar.activation(
                        r2p[:], r1p[:], ACT.Relu,
                        bias=gcol(COL_D21Y, a), scale=-1.0)
                    nc.scalar.activation(
                        s1pv, xpair, ACT.Relu, bias=gcol(COL_NGX1, a))
                    nc.scalar.activation(
                        s2p[:], s1p[:], ACT.Relu,
                        bias=gcol(COL_E21X, a), scale=-1.0)
                    pairs.append((r2p, s2p, False))

                dy = gp.tile([P, FD], F32, tag="dy")
                dx = gp.tile([P, FD], F32, tag="dx")
                for j in range(G):
                    s = slice(j * C, (j + 1) * C)
                    ta, tb, direct = pairs[j]
                    r2v = ta[:].rearrange("p (c j) -> p j c", j=2)
                    s2v = tb[:].rearrange("p (c j) -> p j c", j=2)
                    if direct:
                        # dy = clip(y2) - clip(y1), dx = clip(x2) - clip(x1)
                        nc.vector.tensor_tensor(
                            dy[:, s], r2v[:, 1, :], r2v[:, 0, :], AOP.subtract)
                        nc.vector.tensor_tensor(
                            dx[:, s], s2v[:, 1, :], s2v[:, 0, :], AOP.subtract)
                    else:
                        # dy = r2(y1) - r2(y2), dx = s2(x1) - s2(x2)
                        nc.vector.tensor_tensor(
                            dy[:, s], r2v[:, 0, :], r2v[:, 1, :], AOP.subtract)
                        nc.vector.tensor_tensor(
                            dx[:, s], s2v[:, 0, :], s2v[:, 1, :], AOP.subtract)

                inter = gp.tile([P, FD], F32, tag="inter")
                nc.vector.tensor_tensor(inter[:], dy[:], dx[:], AOP.mult)

                u = gp.tile([P, FD], F32, tag="u")
                dp = gp.tile([P, FD], F32, tag="dp")
                for j in range(G):
                    a = g * G + j
                    s = slice(j * C, (j + 1) * C)
                    nc.vector.scalar_tensor_tensor(
                        u[:, s], area[:], gcol(COL_GA, a), inter[:, s],
                        AOP.add, AOP.subtract)
                nc.vector.scalar_tensor_tensor(
                    dp[:], u[:], 0.7, inter[:], AOP.mult, AOP.subtract)

                m = gp.tile([P, FD], F32, tag="m")
                nc.vector.tensor_tensor(m[:], dp[:], u[:], AOP.mult)
                nc.vector.tensor_tensor(
                    acc[(g + 1) % 2][:], acc[g % 2][:], m[:], AOP.min)

            accfin = acc[NG % 2]
            accv = accfin[:].rearrange("p (a c) -> p c a", a=G)
            macc = pp.tile([P, C], F32, tag="macc")
            nc.vector.tensor_reduce(macc[:], accv, mybir.AxisListType.X, AOP.min)

            lblf = pp.tile([P, C], F32, tag="lblf")
            nc.vector.scalar_tensor_tensor(
                lblf[:], macc[:], 0.0, vb[:], AOP.is_le, AOP.mult)

            oview = outsb[:].rearrange("p (c k) -> p k c", k=2)
            nc.vector.tensor_copy(oview[:, 0, :], lblf[:])
            nc.sync.dma_start(out_d[:], outsb[:])

    nc.finalize()
    return nc


def host_prep(bboxess, gt_bboxess, gt_counts, counts, C):
    """Per-core input shards. Core b gets batch b."""
    B, N, _ = bboxess.shape
    assert N == P * C
    iota = np.arange(N, dtype=np.float32).reshape(P, C)
    in_maps = []
    for b in range(B):
        bb = np.ascontiguousarray(
            bboxess[b].astype(np.float32).reshape(P, 4 * C))
        g = gt_bboxess[b].astype(np.float32).copy()
        nv = int(gt_counts[b, 0])
        g[nv:, 0] = 2.0
        g[nv:, 1] = 2.0
        g[nv:, 2] = 3.0
        g[nv:, 3] = 3.0
        gy1, gx1, gy2, gx2 = g[:, 0], g[:, 1], g[:, 2], g[:, 3]
        row = np.zeros(SCAL_W, dtype=np.float32)
        row[COL_GA:COL_GA + 64] = (gy2 - gy1) * (gx2 - gx1)
        row[COL_NGY1:COL_NGY1 + 64] = -gy1
        row[COL_D21Y:COL_D21Y + 64] = gy2 - gy1
        row[COL_NGX1:COL_NGX1 + 64] = -gx1
        row[COL_E21X:COL_E21X + 64] = gx2 - gx1
        row[COL_GY1:COL_GY1 + 64] = gy1
        row[COL_GY2:COL_GY2 + 64] = gy2
        row[COL_GX1:COL_GX1 + 64] = gx1
        row[COL_GX2:COL_GX2 + 64] = gx2
        row[COL_CNT] = float(int(counts[b, 0]))
        scal = np.ascontiguousarray(np.broadcast_to(row, (P, SCAL_W)))
        in_maps.append({"bbox": bb, "scal": scal, "iota": iota})
    return in_maps


def _axon_reset():
    import ctypes
    try:
        lib = ctypes.CDLL("/opt/axon/libaxon_pjrt.so")
        lib.axon_reset.restype = ctypes.c_int64
        lib.axon_reset()
    except Exception:
        pass


def kernel(bboxess, gt_bboxess, gt_counts, counts):
    B, N, _ = bboxess.shape
    C = N // P
    nc = build_graph(C)
    in_maps = host_prep(bboxess, gt_bboxess, gt_counts, counts, C)
    try:
        res = run_bass_kernel_spmd(nc, in_maps, core_ids=list(range(B)))
    except Exception:
        _axon_reset()
        res = run_bass_kernel_spmd(nc, in_maps, core_ids=list(range(B)))
    out_dtype = np.int64 if counts.dtype == np.int64 else np.int32
    labels = np.empty((B, N, 1), dtype=out_dtype)
    for b in range(B):
        o = res.results[b]["out"]                    # [P, 2C] int32
        pairs = o.reshape(P, C, 2)
        if out_dtype == np.int64:
            labels[b] = pairs.view(np.int64).reshape(N, 1)
        else:
            labels[b] = np.ascontiguousarray(pairs[:, :, 0]).reshape(N, 1)
    return labels



# revision 3
# speedup vs baseline: 2.4404x; 2.4404x over previous
"""Trainium2 Bass kernel for AssignClsLabel (clipped-IoU >= 0.7 proposal labeling).

Problem: bboxess [8, 65536, 4] f32, gt_bboxess [8, 64, 4] f32,
gt_counts/counts [8,1] int. Output labels [8, 65536, 1] int (0/1).

Sharding: work units are (batch, group-of-4-gts) over the FULL proposal
range; only ceil(gt_count/4) groups per batch exist (invalid gts are never
computed). The 45-odd groups are spread over 8 cores (K slots each, padded
with inert far-box slots), so per-core work tracks sum(gt_counts)/8 instead
of 64 gts.

Device math per (proposal n, gt a), all fp32 (validated 0 label flips vs
the jax reference on the fixed dataset):
    yy = clip(y, gy1, gy2) per coord; dy = yy2c - yy1c; dx likewise
    inter = dy*dx
    t1 = (inter - C17*ga) - C17*area        C17 = f32(0.7/1.7)
    t2 = (inter - ga) - area                (= -union)
    pos <=> t1*t2 <= 0                      (<=> iou >= 0.7, sign-exact)
Device outputs w = Sign(t1*t2) as int8 per (slot, gt-lane, n); the host ORs
lanes/slots per batch and applies the n < count mask.

Engine split (per pair-slot, ns): ACT: y-clips in relu form + Sign output;
DVE: x-clips (dual-ALU tensor_scalar, 2x fp32 mode), inter, t1, m;
GPSIMD: dy, dx, t2.
"""
import math
import sys

import numpy as np

if "/opt/trn_rl_repo" not in sys.path:
    sys.path.insert(0, "/opt/trn_rl_repo")

import concourse.mybir as mybir
import concourse.tile as tile
from concourse import bacc
from concourse.bass_utils import run_bass_kernel_spmd

AOP = mybir.AluOpType
ACT = mybir.ActivationFunctionType
F32 = mybir.dt.float32
I8 = mybir.dt.int8

P = 128          # SBUF partitions; proposals n = p*C + c
C = 512          # proposals per partition (N = P*C)
G = 4            # gts per slot
NCORE = 8
C17 = np.float32(np.float32(0.7) / np.float32(1.7))

# scal column layout: per slot s, per gt j, 10 columns
NSC = 10
SC_GY1, SC_GY2, SC_GX1, SC_GX2, SC_GA, SC_GAC17, SC_NGY1, SC_D21Y, SC_NGX1, SC_E21X = range(NSC)

# engine assignment (tunable): 'act' | 'dve' | 'gps'
import os
CFG = {
    "clip_y": "act",   # relu-form (2 ACT passes) or ts dual (dve/gps)
    "clip_x": "dve",
    "dy": os.environ.get("E_DY", "gps"),
    "dx": os.environ.get("E_DX", "gps"),
    "t1": "dve",
    "t2": os.environ.get("E_T2", "gps"),
    "w": os.environ.get("E_W", "act"),
}


def build_graph(K: int, cfg=CFG):
    """SPMD one-core graph with K work slots of G gts each."""
    nc = bacc.Bacc()

    bb_d = nc.declare_dram_parameter("bb", [P, K * 4 * C], F32, isOutput=False)
    ar_d = nc.declare_dram_parameter("ar", [P, K * 2 * C], F32, isOutput=False)
    sc_d = nc.declare_dram_parameter("sc", [P, NSC * G * K], F32, isOutput=False)
    wo_d = nc.declare_dram_parameter("wo", [P, K * G * C], I8, isOutput=True)

    def eng(name):
        e = cfg[name]
        return {"dve": nc.vector, "gps": nc.gpsimd, "act": nc.scalar}[e]

    with tile.TileContext(nc) as tc:
        with (
            tc.tile_pool(name="persist", bufs=1) as pp,
            tc.tile_pool(name="slot", bufs=1) as sp,
        ):
            scal = pp.tile([P, NSC * G * K], F32, tag="scal")
            nc.sync.dma_start(scal[:], sc_d[:])
            # ACT's sync-wait budget is 1: route its scalars through a
            # DVE-produced copy so its data deps collapse onto one proc.
            scal2 = pp.tile([P, NSC * G * K], F32, tag="scal2")
            nc.vector.tensor_copy(scal2[:], scal[:])

            def col(s, j, which):
                c0 = (s * G + j) * NSC + which
                return scal2[:, c0:c0 + 1]

            for s in range(K):
                bbt = sp.tile([P, 4 * C], F32, tag="bb", bufs=2, name=f"bb{s}")
                art = sp.tile([P, 2 * C], F32, tag="ar", bufs=2, name=f"ar{s}")
                nc.sync.dma_start(bbt[:], bb_d[:, s * 4 * C:(s + 1) * 4 * C])
                nc.sync.dma_start(art[:], ar_d[:, s * 2 * C:(s + 1) * 2 * C])
                ypair = bbt[:, 0:2 * C]          # [y1 | y2] blocked
                xpair = bbt[:, 2 * C:4 * C]      # [x1 | x2]
                area = art[:, 0:C]
                areaC17 = art[:, C:2 * C]

                dyg = sp.tile([P, G * C], F32, tag="dyg", bufs=2, name=f"dyg{s}")
                dxg = sp.tile([P, G * C], F32, tag="dxg", bufs=2, name=f"dxg{s}")
                for j in range(G):
                    lane = slice(j * C, (j + 1) * C)
                    # ---- y axis ----
                    if cfg["clip_y"] == "act":
                        r1 = sp.tile([P, 2 * C], F32, tag="r1", bufs=2,
                                     name=f"r1_{s}_{j}")
                        yc = sp.tile([P, 2 * C], F32, tag="yc", bufs=2,
                                     name=f"yc_{s}_{j}")
                        nc.scalar.activation(
                            r1[:], ypair, ACT.Relu, bias=col(s, j, SC_NGY1))
                        nc.scalar.activation(
                            yc[:], r1[:], ACT.Relu,
                            bias=col(s, j, SC_D21Y), scale=-1.0)
                        # relu form: yc = gy2 - clip(y) => dy = yc1 - yc2
                        eng("dy").tensor_tensor(
                            dyg[:, lane], yc[:, 0:C], yc[:, C:2 * C],
                            AOP.subtract)
                    else:
                        yc = sp.tile([P, 2 * C], F32, tag="yc", bufs=2,
                                     name=f"yc_{s}_{j}")
                        eng("clip_y").tensor_scalar(
                            yc[:], ypair, col(s, j, SC_GY1),
                            col(s, j, SC_GY2), AOP.max, AOP.min)
                        eng("dy").tensor_tensor(
                            dyg[:, lane], yc[:, C:2 * C], yc[:, 0:C],
                            AOP.subtract)
                    # ---- x axis ----
                    if cfg["clip_x"] == "act":
                        s1 = sp.tile([P, 2 * C], F32, tag="s1", bufs=2,
                                     name=f"s1_{s}_{j}")
                        xc = sp.tile([P, 2 * C], F32, tag="xc", bufs=2,
                                     name=f"xc_{s}_{j}")
                        nc.scalar.activation(
                            s1[:], xpair, ACT.Relu, bias=col(s, j, SC_NGX1))
                        nc.scalar.activation(
                            xc[:], s1[:], ACT.Relu,
                            bias=col(s, j, SC_E21X), scale=-1.0)
                        eng("dx").tensor_tensor(
                            dxg[:, lane], xc[:, 0:C], xc[:, C:2 * C],
                            AOP.subtract)
                    else:
                        xc = sp.tile([P, 2 * C], F32, tag="xc", bufs=2,
                                     name=f"xc_{s}_{j}")
                        eng("clip_x").tensor_scalar(
                            xc[:], xpair, col(s, j, SC_GX1),
                            col(s, j, SC_GX2), AOP.max, AOP.min)
                        eng("dx").tensor_tensor(
                            dxg[:, lane], xc[:, C:2 * C], xc[:, 0:C],
                            AOP.subtract)

                inter = sp.tile([P, G * C], F32, tag="inter", bufs=2,
                                name=f"inter{s}")
                nc.vector.tensor_tensor(inter[:], dyg[:], dxg[:], AOP.mult)

                t1g = sp.tile([P, G * C], F32, tag="t1g", bufs=2,
                              name=f"t1g{s}")
                t2g = sp.tile([P, G * C], F32, tag="t2g", bufs=2,
                              name=f"t2g{s}")
                for j in range(G):
                    lane = slice(j * C, (j + 1) * C)
                    eng("t1").scalar_tensor_tensor(
                        t1g[:, lane], inter[:, lane], col(s, j, SC_GAC17),
                        areaC17, AOP.subtract, AOP.subtract)
                    eng("t2").scalar_tensor_tensor(
                        t2g[:, lane], inter[:, lane], col(s, j, SC_GA),
                        area, AOP.subtract, AOP.subtract)

                mt = sp.tile([P, G * C], F32, tag="m", bufs=2, name=f"m{s}")
                nc.vector.tensor_tensor(mt[:], t1g[:], t2g[:], AOP.mult)

                wt = sp.tile([P, G * C], I8, tag="w", bufs=2, name=f"w{s}")
                if cfg["w"] == "act":
                    nc.scalar.activation(wt[:], mt[:], ACT.Sign)
                else:
                    nc.vector.tensor_scalar(
                        wt[:], mt[:], 0.0, None, AOP.is_le)
                nc.sync.dma_start(wo_d[:, s * G * C:(s + 1) * G * C], wt[:])

    nc.finalize()
    return nc


FAR = (2.0, 2.0, 3.0, 3.0)  # (gy1, gx1, gy2, gx2) far box: never fires


def plan_work(gt_counts):
    """Work items (batch, a0) -> 8 cores x K slots. Returns (K, slot_map)
    where slot_map[core][s] = (batch, a0) or None for inert pad slots."""
    items = []
    for b in range(len(gt_counts)):
        cnt = int(gt_counts[b])
        for a0 in range(0, cnt, G):
            items.append((b, a0))
    K = max(1, math.ceil(len(items) / NCORE))
    slot_map = []
    for i in range(NCORE):
        sl = items[i * K:(i + 1) * K]
        sl += [None] * (K - len(sl))
        slot_map.append(sl)
    return K, slot_map


def host_prep(bboxess, gt_bboxess, gt_counts, counts, K, slot_map):
    B, N, _ = bboxess.shape
    assert N == P * C
    f32 = np.float32
    # per-batch blocked coord planes and areas
    coords = []
    areas = []
    for b in range(B):
        bb = bboxess[b].astype(f32)                      # [N, 4] y1,x1,y2,x2
        y1 = bb[:, 0].reshape(P, C); x1 = bb[:, 1].reshape(P, C)
        y2 = bb[:, 2].reshape(P, C); x2 = bb[:, 3].reshape(P, C)
        blocked = np.concatenate([y1, y2, x1, x2], axis=1)  # [P, 4C]
        area = ((y2 - y1) * (x2 - x1)).astype(f32)
        areaC17 = (area * C17).astype(f32)
        coords.append(np.ascontiguousarray(blocked))
        areas.append(np.ascontiguousarray(
            np.concatenate([area, areaC17], axis=1)))

    in_maps = []
    for i in range(NCORE):
        bb_arr = np.zeros((P, K * 4 * C), dtype=f32)
        ar_arr = np.zeros((P, K * 2 * C), dtype=f32)
        sc_row = np.zeros(NSC * G * K, dtype=f32)
        for s, item in enumerate(slot_map[i]):
            if item is not None:
                b, a0 = item
                bb_arr[:, s * 4 * C:(s + 1) * 4 * C] = coords[b]
                ar_arr[:, s * 2 * C:(s + 1) * 2 * C] = areas[b]
                cnt = int(gt_counts[b])
            for j in range(G):
                if item is not None and a0 + j < cnt:
                    g = gt_bboxess[b, a0 + j].astype(f32)
                    gy1, gx1, gy2, gx2 = (f32(g[0]), f32(g[1]),
                                          f32(g[2]), f32(g[3]))
                else:
                    gy1, gx1, gy2, gx2 = (f32(FAR[0]), f32(FAR[1]),
                                          f32(FAR[2]), f32(FAR[3]))
                ga = f32(f32(gy2 - gy1) * f32(gx2 - gx1))
                c0 = (s * G + j) * NSC
                sc_row[c0 + SC_GY1] = gy1
                sc_row[c0 + SC_GY2] = gy2
                sc_row[c0 + SC_GX1] = gx1
                sc_row[c0 + SC_GX2] = gx2
                sc_row[c0 + SC_GA] = ga
                sc_row[c0 + SC_GAC17] = f32(C17 * ga)
                sc_row[c0 + SC_NGY1] = -gy1
                sc_row[c0 + SC_D21Y] = f32(gy2 - gy1)
                sc_row[c0 + SC_NGX1] = -gx1
                sc_row[c0 + SC_E21X] = f32(gx2 - gx1)
        sc_arr = np.ascontiguousarray(
            np.broadcast_to(sc_row, (P, NSC * G * K)))
        in_maps.append({"bb": bb_arr, "ar": ar_arr, "sc": sc_arr})
    return in_maps


def merge_output(results, slot_map, counts, K, B, N, out_dtype):
    pos = np.zeros((B, N), dtype=bool)
    for i in range(NCORE):
        w = results[i]["wo"].reshape(P, K, G, C)
        for s, item in enumerate(slot_map[i]):
            if item is None:
                continue
            b, _ = item
            contrib = (w[:, s, :, :] <= 0).any(axis=1)      # [P, C]
            pos[b] |= contrib.reshape(N)
    labels = np.zeros((B, N, 1), dtype=out_dtype)
    for b in range(B):
        nvalid = np.arange(N) < int(counts[b, 0])
        labels[b, :, 0] = (pos[b] & nvalid).astype(out_dtype)
    return labels


def _axon_reset():
    import ctypes
    try:
        lib = ctypes.CDLL("/opt/axon/libaxon_pjrt.so")
        lib.axon_reset.restype = ctypes.c_int64
        lib.axon_reset()
    except Exception:
        pass


def kernel(bboxess, gt_bboxess, gt_counts, counts):
    B, N, _ = bboxess.shape
    K, slot_map = plan_work(np.asarray(gt_counts).reshape(-1))
    nc = build_graph(K)
    in_maps = host_prep(bboxess, gt_bboxess,
                        np.asarray(gt_counts).reshape(-1), counts, K, slot_map)
    try:
        res = run_bass_kernel_spmd(nc, in_maps, core_ids=list(range(NCORE)))
    except Exception:
        _axon_reset()
        res = run_bass_kernel_spmd(nc, in_maps, core_ids=list(range(NCORE)))
    out_dtype = np.int64 if np.asarray(counts).dtype == np.int64 else np.int32
    return merge_output(res.results, slot_map, np.asarray(counts), K, B, N,
                        out_dtype)
